# revision 1
# baseline (speedup 1.0000x reference)
"""Multi-head causal attention (B=4, T=2048, D=1024, H=16) on 8 trn2 NeuronCores.

Sharding: core c handles batch b = c//2 and head-group g = c%2 (8 heads each).
Each core computes Q/K/V projections for its 8 heads, causal attention, and a
row-shard of the output projection; the host sums the two partial outputs per
batch (the "all-reduce") and adds the (b_v @ w_o + b_o) bias term.

Device algebra notes:
  - b_k drops out of softmax entirely (adds a per-query constant to scores).
  - b_v contributes exactly (b_v @ w_o) to every output row -> folded into the
    host-side bias along with b_o.
  - Scores are computed transposed ([k, q] layout) so that softmax sums ride
    free on the AV matmul (ones-column appended to V) and the AV output comes
    out as AV^T, which feeds the w_o matmul with no extra transposes.

DMA rule: hardware DMA descriptors encode at most ONE semaphore wait, so every
DMA destination here is written exactly once (no pool-slot reuse for DMA
targets); partition broadcasts are done with PE outer products, not DMA.
"""

import math
from contextlib import ExitStack

import numpy as np

import concourse.bass as bass
import concourse.mybir as mybir
import concourse.tile as tile
from concourse import bacc
from concourse.bass_utils import run_bass_kernel_spmd


FP = mybir.dt.float32
BF = mybir.dt.bfloat16

D_MODEL = 1024
N_HEADS = 16
B_FULL, T_FULL = 4, 2048
DK = 64                    # head dim
HPC = 8                    # heads per core
DH = HPC * DK              # 512 head-dims per core
N_CORES = 8


def build_bass(seq_len=T_FULL, causal=True, repeat=1, stages='123F'):
    """Build the per-core Bass kernel (SPMD; same NEFF on all 8 cores).

    repeat > 1 wraps the whole kernel in a hardware loop — used only for
    benchmarking (amortizes host dispatch to time the kernel itself).
    """
    T = seq_len
    NT = T // 128             # t-tiles
    NCH = T // 512            # 512-wide t/q chunks
    ND = D_MODEL // 128       # d_model tiles (8)
    NM = DH // 128            # head-pair tiles (4)

    nc = bacc.Bacc("TRN2", target_bir_lowering=False, debug=False)
    # inputs arrive pre-cast to bf16 (host does the fp32->bf16 conversion)
    xbf_d = nc.dram_tensor("xbf", [T, D_MODEL], BF, kind="ExternalInput")
    wq_d = nc.dram_tensor("wq", [D_MODEL, DH], BF, kind="ExternalInput")
    wk_d = nc.dram_tensor("wk", [D_MODEL, DH], BF, kind="ExternalInput")
    wv_d = nc.dram_tensor("wv", [D_MODEL, DH], BF, kind="ExternalInput")
    wo_d = nc.dram_tensor("wo", [DH, D_MODEL], BF, kind="ExternalInput")
    bq_d = nc.dram_tensor("bq", [128, DH // 128], FP, kind="ExternalInput")
    out_d = nc.dram_tensor("out", [T, D_MODEL], FP, kind="ExternalOutput")

    with ExitStack() as ctx:
        tc = ctx.enter_context(tile.TileContext(nc))
        persist = ctx.enter_context(tc.tile_pool(name="persist", bufs=1))
        qt_pool = ctx.enter_context(tc.tile_pool(name="qt", bufs=3))
        xt_pool = ctx.enter_context(tc.tile_pool(name="xt", bufs=3))
        at_pool = ctx.enter_context(tc.tile_pool(name="atp", bufs=6))
        rec_pool = ctx.enter_context(tc.tile_pool(name="rec", bufs=2))
        avn_pool = ctx.enter_context(tc.tile_pool(name="avn", bufs=2))
        out_pool = ctx.enter_context(tc.tile_pool(name="outp", bufs=3))
        mm_ps = ctx.enter_context(tc.tile_pool(name="mmps", bufs=2, space="PSUM"))
        sc_ps = ctx.enter_context(tc.tile_pool(name="scps", bufs=2, space="PSUM"))
        av_ps = ctx.enter_context(tc.tile_pool(name="avps", bufs=2, space="PSUM"))

        def emit_kernel():
            # ---- constants -------------------------------------------------
            ones_bf = persist.tile([1, DK], BF, name="ones_bf", tag="ones_bf")
            nc.gpsimd.memset(ones_bf, 1.0)

            masks = []
            if causal:
                for j in range(4):
                    m = persist.tile([128, 512], BF, name=f"mask{j}", tag=f"mask{j}")
                    nc.gpsimd.memset(m, 1.0)
                    # keep where (q - k - 128*j) >= 0, else 0
                    nc.gpsimd.affine_select(
                        out=m, in_=m, compare_op=mybir.AluOpType.is_ge,
                        fill=0.0, base=-128 * j, pattern=[[1, 512]],
                        channel_multiplier=-1)
                    masks.append(m)

            def bcast_mid(ap, nmid):
                return bass.AP(tensor=ap.tensor, offset=ap.offset,
                               ap=[list(ap.ap[0]), [0, nmid], list(ap.ap[1])])

            # ---- weights: one 3D-AP HWDGE DMA per matrix (host pre-cast) ---
            def load_weight_bf(dram, n_tiles, cols, label):
                wsb = persist.tile([128, n_tiles, cols], BF, name=f"w_{label}",
                                   tag=f"w_{label}")
                nc.sync.dma_start(
                    out=wsb,
                    in_=dram.ap().rearrange("(j p) c -> p j c", p=128))
                return [wsb[:, j, :] for j in range(n_tiles)]

            wk_bf = load_weight_bf(wk_d, ND, DH, "k")
            wq_bf = load_weight_bf(wq_d, ND, DH, "q")
            wv_bf = load_weight_bf(wv_d, ND, DH, "v")
            wo_bf = load_weight_bf(wo_d, NM, D_MODEL, "o")

            bq_sb = persist.tile([128, NM], FP, name="bq_sb", tag="bq_sb")
            nc.sync.dma_start(out=bq_sb, in_=bq_d[:, :])

            # ---- persistent per-core tensors ------------------------------
            # KT[mt]: [128, T] bf16, rows = head-dim (pair mt: heads 2mt,2mt+1)
            KT = [persist.tile([128, T], BF, name=f"KT{mt}", tag=f"KT{mt}")
                  for mt in range(NM)]
            # V natural layout with ones column: [128 t, 8 heads, 64+1]
            V_sb = [persist.tile([128, HPC, DK + 1], BF, name=f"V{i}", tag=f"V{i}")
                    for i in range(NT)]
            for i in range(NT):
                nc.gpsimd.memset(V_sb[i][:, :, DK], 1.0)
            # AV^T, write-once (DMA target for the h1 partition shift)
            AVT_all = [[persist.tile([128, 512], BF, name=f"AVT{n}_{p}",
                                     tag=f"AVT{n}_{p}") for p in range(NM)]
                       for n in range(NCH)]

            # ---- main streamed loop over 512-wide chunks -------------------
            # S2(n+1) and F(n-1) matmul groups are fed into S3(n)'s pair
            # boundaries: the per-engine queues are in-order, so S3's
            # exp-latency bubbles can only be filled by positionally
            # interleaving independent work into its instruction stream.

            def emit_s1(n):
                xT = [xt_pool.tile([128, 512], BF, name=f"xT{j}",
                                   tag=f"xT{j}") for j in range(ND)]
                for j in range(ND):
                    nc.sync.dma_start(out=xT[j],
                                      in_=xbf_d[n * 512:(n + 1) * 512,
                                                j * 128:(j + 1) * 128],
                                      transpose=True)
                return xT

            def make_s2_groups(n, xT):
                csl = bass.ts(n, 512)
                QT = [qt_pool.tile([128, 512], BF, name=f"QT{mt}",
                                   tag=f"QT{mt}") for mt in range(NM)]
                groups = []

                def qgroup(mt):
                    msl = bass.ts(mt, 128)
                    ps = mm_ps.tile([128, 512], FP, name="psq", tag="mm")
                    for j in range(ND):
                        nc.tensor.matmul(ps, wq_bf[j][:, msl], xT[j],
                                         start=(j == 0), stop=(j == ND - 1))
                    nc.vector.tensor_scalar_add(QT[mt], ps, bq_sb[:, mt:mt + 1])

                def kgroup(mt):
                    msl = bass.ts(mt, 128)
                    ps = mm_ps.tile([128, 512], FP, name="psk", tag="mm")
                    for j in range(ND):
                        nc.tensor.matmul(ps, wk_bf[j][:, msl], xT[j],
                                         start=(j == 0), stop=(j == ND - 1))
                    # fold the 1/sqrt(dk) score scale into K^T so exp needs
                    # no scale and score magnitudes stay ~N(0,1)
                    nc.vector.tensor_scalar_mul(KT[mt][:, csl], ps,
                                                1.0 / math.sqrt(DK))

                def vgroup(il):
                    i = 4 * n + il
                    ps = mm_ps.tile([128, 512], FP, name="psv", tag="mm")
                    for j in range(ND):
                        nc.tensor.matmul(ps, xT[j][:, il * 128:(il + 1) * 128],
                                         wv_bf[j], start=(j == 0),
                                         stop=(j == ND - 1))
                    nc.vector.tensor_copy(
                        V_sb[i][:, :, 0:DK],
                        ps.rearrange("p (h d) -> p h d", h=HPC))

                if '2' in stages:
                    for mt in range(NM):
                        groups.append(lambda mt=mt: kgroup(mt))
                        groups.append(lambda mt=mt: qgroup(mt))
                    for il in range(4):
                        groups.append(lambda il=il: vgroup(il))
                return QT, groups

            def make_f_groups(n):
                AVT = AVT_all[n]
                groups = []

                osbs = {}

                def fgroup(il, cc):
                    i = 4 * n + il
                    isl = bass.ts(il, 128)
                    if il not in osbs:
                        osbs[il] = out_pool.tile([128, D_MODEL], FP,
                                                 name="osb", tag="osb")
                    osb = osbs[il]
                    ps = mm_ps.tile([128, 512], FP, name="pso", tag="mm")
                    for dk in range(NM):
                        nc.tensor.matmul(
                            ps, AVT[dk][:, isl],
                            wo_bf[dk][:, cc * 512:(cc + 1) * 512],
                            start=(dk == 0), stop=(dk == NM - 1))
                    nc.vector.tensor_copy(osb[:, cc * 512:(cc + 1) * 512], ps)
                    if cc == 1:
                        nc.sync.dma_start(
                            out=out_d[i * 128:(i + 1) * 128, :], in_=osb)

                if 'F' in stages:
                    for il in range(4):
                        for cc in range(2):
                            groups.append(lambda il=il, cc=cc: fgroup(il, cc))
                return groups

            def emit_s3(n, QT, feed):
                AVT = AVT_all[n]
                nkt = 4 * n + 4 if causal else NT
                PIPE = 3
                pending_norm = [None]

                def emit_norm(p, av0, av1):
                    den_bf = rec_pool.tile([1, 2, 512], BF, name="den_bf",
                                           tag="den_bf")
                    nc.vector.tensor_copy(den_bf[:, 0, :], av0[DK:DK + 1, :])
                    nc.vector.tensor_copy(den_bf[:, 1, :], av1[DK:DK + 1, :])
                    rb_sb = rec_pool.tile([DK, 2, 512], FP, name="rb_sb",
                                          tag="rb_sb")
                    for hh in range(2):
                        rb = mm_ps.tile([DK, 512], FP, name=f"rb{hh}",
                                        tag="mm")
                        nc.tensor.matmul(rb, ones_bf, den_bf[:, hh, :],
                                         start=True, stop=True)
                        nc.vector.reciprocal(rb_sb[:, hh, :], rb)
                    nc.vector.tensor_mul(AVT[p][0:64, :], av0[0:DK, :],
                                         rb_sb[:, 0, :])
                    avn1 = avn_pool.tile([64, 512], BF, name="avn1",
                                         tag="avn1")
                    nc.vector.tensor_mul(avn1, av1[0:DK, :], rb_sb[:, 1, :])
                    nc.sync.dma_start(out=AVT[p][64:128, :], in_=avn1)

                npairs = NM if '3' in stages else 0
                for p in range(npairs):
                    av0 = av_ps.tile([DK + 1, 512], FP, name="av0", tag="av")
                    av1 = av_ps.tile([DK + 1, 512], FP, name="av1", tag="av")
                    avs = (av0, av1)

                    def emit_av(kt, at, q0):
                        for hh in range(2):
                            nc.tensor.matmul(
                                avs[hh][:, q0:512],
                                V_sb[kt][:, 2 * p + hh, :],
                                at[:, hh, q0:512],
                                start=(kt == 0), stop=(kt == nkt - 1),
                                skip_group_check=True)

                    pend = []
                    for kt in range(nkt):
                        ksl = bass.ts(kt, 128)
                        # columns < j*128 of a diagonal tile are fully
                        # masked: skip in scores matmul / exp / mask / AV
                        j = kt - 4 * n if (causal and kt >= 4 * n) else 0
                        q0 = j * 128
                        ps_s = sc_ps.tile([128, 2, 512], FP, name="ps_s",
                                          tag="sc")
                        at = at_pool.tile([128, 2, 512], BF, name="at",
                                          tag="at")
                        for hh in range(2):
                            nc.tensor.matmul(
                                ps_s[:, hh, q0:512],
                                KT[p][hh * 64:(hh + 1) * 64, ksl],
                                QT[p][hh * 64:(hh + 1) * 64, q0:512],
                                start=True, stop=True,
                                tile_position=(hh * 64, 0))
                        nc.scalar.activation(at[:, :, q0:512],
                                             ps_s[:, :, q0:512],
                                             mybir.ActivationFunctionType.Exp)
                        if causal and kt >= 4 * n:
                            nc.vector.tensor_mul(
                                at[:, :, q0:512], at[:, :, q0:512],
                                bcast_mid(masks[kt - 4 * n][:, q0:512], 2))
                        if kt == 0 and pending_norm[0] is not None:
                            pending_norm[0]()
                            pending_norm[0] = None
                        pend.append((kt, at, q0))
                        if len(pend) > PIPE:
                            emit_av(*pend.pop(0))
                    for item in pend:
                        emit_av(*item)
                    pending_norm[0] = (lambda p=p, a0=av0, a1=av1:
                                       emit_norm(p, a0, a1))
                    # fill the pair-transition bubble with independent work
                    for _ in range(5):
                        if feed:
                            feed.pop(0)()
                if pending_norm[0] is not None:
                    pending_norm[0]()
                    pending_norm[0] = None
                while feed:
                    feed.pop(0)()

            xT_cur = emit_s1(0)
            QT_cur, s2g = make_s2_groups(0, xT_cur)
            for g in s2g:
                g()
            f_prev = []
            for n in range(NCH):
                feed = []
                if n + 1 < NCH:
                    xT_next = emit_s1(n + 1)
                    QT_next, s2g_next = make_s2_groups(n + 1, xT_next)
                    feed += s2g_next
                else:
                    QT_next = None
                feed += f_prev
                if '3' in stages:
                    emit_s3(n, QT_cur, feed)
                else:
                    for g in feed:
                        g()
                f_prev = make_f_groups(n)
                QT_cur = QT_next
            for g in f_prev:
                g()

        if repeat > 1:
            with tc.For_i(0, repeat, 1):
                emit_kernel()
        else:
            emit_kernel()

    nc.compile()
    return nc


_NC_CACHE = {}


def _get_nc(seq_len, causal):
    key = (seq_len, causal)
    if key not in _NC_CACHE:
        _NC_CACHE[key] = build_bass(seq_len, causal)
    return _NC_CACHE[key]


def make_in_maps(x, w_q, b_q, w_k, w_v, w_o):
    """Per-core input dicts for the 8 cores (weights/x pre-cast to bf16)."""
    import ml_dtypes
    bf = ml_dtypes.bfloat16
    x_bf = np.ascontiguousarray(x).astype(bf)
    wq_bf = w_q.astype(bf)
    wk_bf = w_k.astype(bf)
    wv_bf = w_v.astype(bf)
    wo_bf = w_o.astype(bf)
    in_maps = []
    for c in range(N_CORES):
        b, g = divmod(c, 2)
        sl = slice(g * DH, (g + 1) * DH)
        in_maps.append({
            "xbf": x_bf[b],
            "wq": np.ascontiguousarray(wq_bf[:, sl]),
            "wk": np.ascontiguousarray(wk_bf[:, sl]),
            "wv": np.ascontiguousarray(wv_bf[:, sl]),
            "wo": np.ascontiguousarray(wo_bf[sl, :]),
            "bq": np.ascontiguousarray(
                b_q[sl].reshape(DH // 128, 128).T.astype(np.float32)),
        })
    return in_maps


def kernel(x, mask, w_q, b_q, w_k, b_k, w_v, b_v, w_o, b_o, _trace=False):
    x = np.asarray(x, dtype=np.float32)
    mask_np = np.asarray(mask).reshape(mask.shape[-2], mask.shape[-1])
    w_q, b_q = np.asarray(w_q, np.float32), np.asarray(b_q, np.float32)
    w_k = np.asarray(w_k, np.float32)
    w_v, b_v = np.asarray(w_v, np.float32), np.asarray(b_v, np.float32)
    w_o, b_o = np.asarray(w_o, np.float32), np.asarray(b_o, np.float32)

    T = x.shape[1]
    tril = np.tril(np.ones((T, T), dtype=mask_np.dtype))
    if np.array_equal(mask_np, tril):
        causal = True
    elif np.all(mask_np != 0):
        causal = False
    else:
        raise NotImplementedError("only causal or all-ones masks supported")

    nc = _get_nc(T, causal)
    in_maps = make_in_maps(x, w_q, b_q, w_k, w_v, w_o)
    res = run_bass_kernel_spmd(nc, in_maps, core_ids=list(range(N_CORES)),
                               trace=_trace)

    host_bias = (b_v @ w_o + b_o).astype(np.float32)
    out = np.empty((x.shape[0], T, D_MODEL), dtype=np.float32)
    for b in range(x.shape[0]):
        out[b] = res.results[2 * b]["out"] + res.results[2 * b + 1]["out"] \
            + host_bias
    kernel._last_result = res
    return out



# revision 2
# speedup vs baseline: 1.0184x; 1.0184x over previous
"""Multi-head causal attention (B=4, T=2048, D=1024, H=16) on 8 trn2 NeuronCores.

Sharding: core c handles batch b = c//2 and head-group g = c%2 (8 heads each).
Each core computes Q/K/V projections for its 8 heads, causal attention, and a
row-shard of the output projection; the host sums the two partial outputs per
batch (the "all-reduce") and adds the (b_v @ w_o + b_o) bias term.

Device algebra notes:
  - b_k drops out of softmax entirely (adds a per-query constant to scores).
  - b_v contributes exactly (b_v @ w_o) to every output row -> folded into the
    host-side bias along with b_o.
  - Scores are computed transposed ([k, q] layout) so that softmax sums ride
    free on the AV matmul (ones-column appended to V) and the AV output comes
    out as AV^T, which feeds the w_o matmul with no extra transposes.
  - x is shipped pre-transposed/pre-tiled from the host ([128, 8, T] bf16) so
    every device DMA is a clean 3D-AP contiguous transfer (no DMA transpose,
    which also removes the transpose||SBUF-SBUF DMA serialization hazard).
  - K^T is stored zero-padded per head-half (kpad) so the scores matmuls run
    with a full K=128 contraction (FWL-eligible weight loads) instead of two
    K=64 loads.
  - Softmax normalization: reciprocal on the 1-partition denominator row
    first, then a PE ones-outer-product broadcasts the reciprocal; the
    PSUM->SBUF bounce rides on the scalar engine, which has slack.
  - Independent projection/output-proj matmul work is paced into the
    attention inner loop in ~1us chunks (pace_tiles) so no engine sees a
    multi-us burst of foreign work.

DMA rule: hardware DMA descriptors encode at most ONE semaphore wait, so every
DMA destination here is written exactly once per iteration; partition
broadcasts are done with PE outer products, not DMA.
"""

import math
from contextlib import ExitStack

import numpy as np

import concourse.bass as bass
import concourse.mybir as mybir
import concourse.tile as tile
from concourse import bacc
from concourse.bass_utils import run_bass_kernel_spmd


FP = mybir.dt.float32
BF = mybir.dt.bfloat16

D_MODEL = 1024
N_HEADS = 16
B_FULL, T_FULL = 4, 2048
DK = 64                    # head dim
HPC = 8                    # heads per core
DH = HPC * DK              # 512 head-dims per core
N_CORES = 8


def build_bass(seq_len=T_FULL, causal=True, repeat=1, stages='123F',
               pipe=3, do_mask=True, at_bufs=6, micro=None, micro_n=512,
               rb_on_act=True, half_exp=False, pace_tiles=4, kpad=True):
    """Build the per-core Bass kernel (SPMD; same NEFF on all 8 cores).

    repeat > 1 wraps the whole kernel in a hardware loop — used only for
    benchmarking (amortizes host dispatch to time the kernel itself).
    """
    T = seq_len
    NT = T // 128             # t-tiles
    NCH = T // 512            # 512-wide t/q chunks
    ND = D_MODEL // 128       # d_model tiles (8)
    NM = DH // 128            # head-pair tiles (4)

    nc = bacc.Bacc("TRN2", target_bir_lowering=False, debug=False)
    # x arrives pre-transposed + pre-tiled on host: [128, ND, T] bf16 where
    # xt[p, j, t] = x[t, j*128 + p]
    xbf_d = nc.dram_tensor("xbf", [128, ND, T], BF, kind="ExternalInput")
    wq_d = nc.dram_tensor("wq", [D_MODEL, DH], BF, kind="ExternalInput")
    wk_d = nc.dram_tensor("wk", [D_MODEL, DH], BF, kind="ExternalInput")
    wv_d = nc.dram_tensor("wv", [D_MODEL, DH], BF, kind="ExternalInput")
    wo_d = nc.dram_tensor("wo", [DH, D_MODEL], BF, kind="ExternalInput")
    bq_d = nc.dram_tensor("bq", [128, DH // 128], FP, kind="ExternalInput")
    out_d = nc.dram_tensor("out", [T, D_MODEL], FP, kind="ExternalOutput")

    with ExitStack() as ctx:
        tc = ctx.enter_context(tile.TileContext(nc))
        persist = ctx.enter_context(tc.tile_pool(name="persist", bufs=1))
        qt_pool = ctx.enter_context(tc.tile_pool(name="qt", bufs=3))
        xt_pool = ctx.enter_context(tc.tile_pool(name="xt", bufs=3))
        at_pool = ctx.enter_context(tc.tile_pool(name="atp", bufs=at_bufs))
        rec_pool = ctx.enter_context(tc.tile_pool(name="rec", bufs=2))
        avn_pool = ctx.enter_context(tc.tile_pool(name="avn", bufs=2))
        out_pool = ctx.enter_context(tc.tile_pool(name="outp", bufs=3))
        mm_ps = ctx.enter_context(tc.tile_pool(name="mmps", bufs=2, space="PSUM"))
        sc_ps = ctx.enter_context(tc.tile_pool(name="scps", bufs=2, space="PSUM"))
        av_ps = ctx.enter_context(tc.tile_pool(name="avps", bufs=2, space="PSUM"))

        def emit_kernel():
            # ---- constants -------------------------------------------------
            ones_bf = persist.tile([1, DK], BF, name="ones_bf", tag="ones_bf")
            nc.gpsimd.memset(ones_bf, 1.0)

            masks = []
            if causal:
                # one [128,128] lower-triangle tile: keep where col >= row.
                # Every diagonal k-tile needs masking only on its first 128
                # columns (cols beyond are fully unmasked, earlier cols are
                # fully masked and already skipped via q0).
                m = persist.tile([128, 128], BF, name="mask01", tag="mask01")
                nc.gpsimd.memset(m, 1.0)
                nc.gpsimd.affine_select(
                    out=m, in_=m, compare_op=mybir.AluOpType.is_ge,
                    fill=0.0, base=0, pattern=[[1, 128]],
                    channel_multiplier=-1)
                masks = [m] * 4

            def bcast_mid(ap, nmid):
                return bass.AP(tensor=ap.tensor, offset=ap.offset,
                               ap=[list(ap.ap[0]), [0, nmid], list(ap.ap[1])])

            # x chunk 0 DMA first so S2(0) can start as early as possible
            xt0 = xt_pool.tile([128, ND, 512], BF, name="xTt", tag="xTt")
            nc.sync.dma_start(out=xt0, in_=xbf_d[:, :, 0:512])
            xT0 = [xt0[:, j, :] for j in range(ND)]

            # ---- weights: one 3D-AP HWDGE DMA per matrix (host pre-cast) ---
            def load_weight_bf(dram, n_tiles, cols, label):
                wsb = persist.tile([128, n_tiles, cols], BF, name=f"w_{label}",
                                   tag=f"w_{label}")
                nc.sync.dma_start(
                    out=wsb,
                    in_=dram.ap().rearrange("(j p) c -> p j c", p=128))
                return [wsb[:, j, :] for j in range(n_tiles)]

            wk_bf = load_weight_bf(wk_d, ND, DH, "k")
            wq_bf = load_weight_bf(wq_d, ND, DH, "q")
            wv_bf = load_weight_bf(wv_d, ND, DH, "v")

            bq_sb = persist.tile([128, NM], FP, name="bq_sb", tag="bq_sb")
            nc.sync.dma_start(out=bq_sb, in_=bq_d[:, :])

            wo_bf = load_weight_bf(wo_d, NM, D_MODEL, "o")

            # ---- persistent per-core tensors ------------------------------
            # KT[mt]: [128, T] bf16, rows = head-dim (pair mt: heads 2mt,2mt+1)
            # kpad: per head-half tiles with the other 64 rows zeroed, so the
            # scores matmuls are full K=128 (FWL-eligible; no K=64 LDW
            # penalty). rhs is then the full 128-row QT (the other head's
            # rows meet zero weights).
            if kpad:
                KTp = [[persist.tile([128, T], BF, name=f"KT{mt}h{hh}",
                                     tag=f"KT{mt}h{hh}") for hh in range(2)]
                       for mt in range(NM)]
                for mt in range(NM):
                    nc.gpsimd.memset(KTp[mt][0][64:128, :], 0.0)
                    nc.gpsimd.memset(KTp[mt][1][0:64, :], 0.0)
                KT = None
            else:
                KT = [persist.tile([128, T], BF, name=f"KT{mt}",
                                   tag=f"KT{mt}") for mt in range(NM)]
            # V natural layout with ones column: [128 t, 8 heads, 64+1]
            V_sb = [persist.tile([128, HPC, DK + 1], BF, name=f"V{i}", tag=f"V{i}")
                    for i in range(NT)]
            for i in range(NT):
                nc.gpsimd.memset(V_sb[i][:, :, DK], 1.0)
            # AV^T, write-once (DMA target for the h1 partition shift)
            AVT_all = [[persist.tile([128, 512], BF, name=f"AVT{n}_{p}",
                                     tag=f"AVT{n}_{p}") for p in range(NM)]
                       for n in range(NCH)]

            def emit_micro():
                """Microbench streams using the real kernel's tiles/pools."""
                if micro in ("mm_same_w", "mm_alt_w"):
                    pss = [mm_ps.tile([128, 512], FP, name=f"psu{i}",
                                      tag="mm") for i in range(2)]
                    for i in range(micro_n):
                        j = 0 if micro == "mm_same_w" else i % ND
                        nc.tensor.matmul(pss[i % 2], wq_bf[j][:, 0:128],
                                         wq_bf[1], start=True, stop=True,
                                         skip_group_check=True)
                    consume = pss[0][:, 0:512]
                elif micro in ("mm_k64_pair", "mm_k64_serial"):
                    sss = [sc_ps.tile([128, 2, 512], FP, name=f"ps_su{i}",
                                      tag="sc") for i in range(2)]
                    for i in range(micro_n // 2):
                        for hh in range(2):
                            if micro == "mm_k64_pair":
                                nc.tensor.matmul(
                                    sss[i % 2][:, hh, :],
                                    wk_bf[0][hh * 64:(hh + 1) * 64, 0:128],
                                    wk_bf[1][hh * 64:(hh + 1) * 64, 0:512],
                                    start=True, stop=True,
                                    skip_group_check=True,
                                    tile_position=(hh * 64, 0))
                            else:
                                nc.tensor.matmul(
                                    sss[i % 2][:, hh, :],
                                    wk_bf[0][0:64, 0:128],
                                    wk_bf[1][0:64, 0:512],
                                    start=True, stop=True,
                                    skip_group_check=True,
                                    tile_position=(0, 0))
                    consume = sss[0][:, 0, :]
                elif micro == "mm_n1024":
                    ps2b = [sc_ps.tile([128, 2, 512], FP, name=f"psw{i}",
                                       tag="sc") for i in range(2)]
                    for i in range(micro_n // 2):
                        nc.tensor.matmul(
                            ps2b[i % 2].rearrange("p a b -> p (a b)"),
                            wq_bf[i % ND][:, 0:128], wo_bf[0],
                            start=True, stop=True, skip_group_check=True)
                    consume = ps2b[0][:, 0, :]
                elif micro == "mm_m65":
                    avs = [av_ps.tile([DK + 1, 512], FP, name=f"avu{i}",
                                      tag="av") for i in range(2)]
                    for i in range(micro_n):
                        nc.tensor.matmul(avs[i % 2], wv_bf[0][:, 0:65],
                                         wk_bf[1][:, 0:512],
                                         start=True, stop=True,
                                         skip_group_check=True)
                    consume = avs[0][0:64, :]
                elif micro in ("act_exp", "act_exp1", "dve_copy_ps"):
                    ps_s = sc_ps.tile([128, 2, 512], FP, name="ps_su",
                                      tag="sc")
                    at = at_pool.tile([128, 2, 512], BF, name="atu",
                                      tag="at")
                    for hh in range(2):
                        nc.tensor.matmul(ps_s[:, hh, :], wq_bf[0][:, 0:128],
                                         wq_bf[1], start=True, stop=True,
                                         skip_group_check=True)
                    for i in range(micro_n):
                        if micro == "act_exp":
                            nc.scalar.activation(
                                at, ps_s, mybir.ActivationFunctionType.Exp)
                        elif micro == "act_exp1":
                            nc.scalar.activation(
                                at[:, 0, :], ps_s[:, 0, :],
                                mybir.ActivationFunctionType.Exp)
                        else:
                            nc.vector.tensor_copy(at[:, 0, :], ps_s[:, 0, :])
                    consume = at[:, 0, 0:512]
                elif micro == "dve_mul":
                    at = at_pool.tile([128, 2, 512], BF, name="atu",
                                      tag="at")
                    at2 = at_pool.tile([128, 2, 512], BF, name="atu2",
                                      tag="at")
                    nc.gpsimd.memset(at2, 1.0)
                    for i in range(micro_n):
                        nc.vector.tensor_mul(
                            at, at2, bcast_mid(masks[0], 2))
                    consume = at[:, 0, 0:512]
                else:
                    raise ValueError(micro)
                osb = out_pool.tile([128, D_MODEL], FP, name="osb",
                                    tag="osb")
                npart = consume.ap[0][1] if hasattr(consume, "ap") else 128
                nc.vector.tensor_copy(osb[0:npart, 0:512], consume)
                nc.sync.dma_start(out=out_d[0:128, :], in_=osb)

            # ---- main streamed loop over 512-wide chunks -------------------
            # S2(n+1) and F(n-1) matmul groups are fed into S3(n)'s pair
            # boundaries: the per-engine queues are in-order, so S3's
            # exp-latency bubbles can only be filled by positionally
            # interleaving independent work into its instruction stream.

            def emit_s1(n):
                xt = xt_pool.tile([128, ND, 512], BF, name="xTt", tag="xTt")
                nc.sync.dma_start(out=xt,
                                  in_=xbf_d[:, :, n * 512:(n + 1) * 512])
                return [xt[:, j, :] for j in range(ND)]

            def make_s2_parts(n, xT):
                """Projection work for chunk n as (est_ns, emit_fn) parts.

                Each q/k/v group (8 accumulating MMs + a vector op) is split
                into two 4-MM halves so the S3 pacer can interleave at ~1us
                granularity. The halves share one PSUM tile via `state`.
                """
                csl = bass.ts(n, 512)
                QT = [qt_pool.tile([128, 512], BF, name=f"QT{mt}",
                                   tag=f"QT{mt}") for mt in range(NM)]
                parts = []
                state = {}

                def half(kind, mt, j0):
                    msl = bass.ts(mt, 128)
                    if j0 == 0:
                        ps = mm_ps.tile([128, 512], FP, name=f"ps{kind}",
                                        tag="mm")
                        state[(kind, mt)] = ps
                    else:
                        ps = state.pop((kind, mt))
                    for j in range(j0, j0 + 4):
                        if kind == 'v':
                            nc.tensor.matmul(
                                ps, xT[j][:, mt * 128:(mt + 1) * 128],
                                wv_bf[j], start=(j == 0),
                                stop=(j == ND - 1))
                        else:
                            w = wq_bf if kind == 'q' else wk_bf
                            nc.tensor.matmul(ps, w[j][:, msl], xT[j],
                                             start=(j == 0),
                                             stop=(j == ND - 1))
                    if j0 + 4 < ND:
                        return
                    if kind == 'q':
                        nc.vector.tensor_scalar_add(QT[mt], ps,
                                                    bq_sb[:, mt:mt + 1])
                    elif kind == 'k':
                        # fold the 1/sqrt(dk) score scale into K^T
                        sc = 1.0 / math.sqrt(DK)
                        if kpad:
                            nc.vector.tensor_scalar_mul(
                                KTp[mt][0][0:64, csl], ps[0:64, :], sc)
                            nc.vector.tensor_scalar_mul(
                                KTp[mt][1][64:128, csl], ps[64:128, :], sc)
                        else:
                            nc.vector.tensor_scalar_mul(KT[mt][:, csl], ps,
                                                        sc)
                    else:
                        i = 4 * n + mt
                        nc.vector.tensor_copy(
                            V_sb[i][:, :, 0:DK],
                            ps.rearrange("p (h d) -> p h d", h=HPC))

                if '2' in stages:
                    for mt in range(NM):
                        for kind in ('k', 'q'):
                            for j0 in (0, 4):
                                parts.append((990, lambda kind=kind, mt=mt,
                                              j0=j0: half(kind, mt, j0)))
                    for il in range(4):
                        for j0 in (0, 4):
                            parts.append((990, lambda il=il, j0=j0:
                                          half('v', il, j0)))
                return QT, parts

            def make_f_parts(n):
                AVT = AVT_all[n]
                parts = []
                osbs = {}

                def fgroup(il, cc):
                    i = 4 * n + il
                    isl = bass.ts(il, 128)
                    if il not in osbs:
                        osbs[il] = out_pool.tile([128, D_MODEL], FP,
                                                 name="osb", tag="osb")
                    osb = osbs[il]
                    ps = mm_ps.tile([128, 512], FP, name="pso", tag="mm")
                    for dk in range(NM):
                        nc.tensor.matmul(
                            ps, AVT[dk][:, isl],
                            wo_bf[dk][:, cc * 512:(cc + 1) * 512],
                            start=(dk == 0), stop=(dk == NM - 1))
                    nc.vector.tensor_copy(osb[:, cc * 512:(cc + 1) * 512], ps)
                    if cc == 1:
                        nc.sync.dma_start(
                            out=out_d[i * 128:(i + 1) * 128, :], in_=osb)

                if 'F' in stages:
                    for il in range(4):
                        for cc in range(2):
                            parts.append((1030, lambda il=il, cc=cc:
                                          fgroup(il, cc)))
                return parts

            def emit_s3(n, QT, feed):
                AVT = AVT_all[n]
                nkt = 4 * n + 4 if causal else NT
                PIPE = pipe
                pending_norm = [None]
                npairs = NM if '3' in stages else 0

                # proportional pacer: spread the independent S2/F matmul work
                # evenly across this chunk's kt tiles so PE never idles while
                # ACT streams exps, and ACT never idles through a feed burst.
                feed_total = sum(c for c, _ in feed)
                total_tiles = max(1, nkt * max(npairs, 1))
                done = [0.0]
                tcount = [0]

                def pace():
                    tcount[0] += 1
                    if pace_tiles > 0 and tcount[0] % pace_tiles != 0:
                        return
                    target = feed_total * tcount[0] / total_tiles
                    while feed and done[0] < target:
                        c, g = feed.pop(0)
                        g()
                        done[0] += c

                def emit_norm(p, av0, av1):
                    # reciprocal on the 1-partition denominator rows, then a
                    # PE ones-outer-product broadcast of the RECIPROCAL, and
                    # an ACT copy bounce PSUM->SBUF (DVE is the scarcer
                    # engine; ACT has slack).
                    rec_sb = rec_pool.tile([1, 2, 512], BF, name="rec_sb",
                                           tag="den_bf")
                    with nc.allow_low_precision(
                            reason="bf16 1/den adds ~0.4% to the softmax "
                                   "denominator, well inside the error "
                                   "budget"):
                        nc.vector.reciprocal(rec_sb[:, 0, :],
                                             av0[DK:DK + 1, :])
                        nc.vector.reciprocal(rec_sb[:, 1, :],
                                             av1[DK:DK + 1, :])
                    rb_sb = rec_pool.tile([DK, 2, 512], BF, name="rb_sb",
                                          tag="rb_sb")
                    for hh in range(2):
                        rb = mm_ps.tile([DK, 512], FP, name=f"rb{hh}",
                                        tag="mm")
                        nc.tensor.matmul(rb, ones_bf, rec_sb[:, hh, :],
                                         start=True, stop=True)
                        if rb_on_act:
                            nc.scalar.copy(rb_sb[:, hh, :], rb)
                        else:
                            nc.vector.tensor_copy(rb_sb[:, hh, :], rb)
                    nc.vector.tensor_mul(AVT[p][0:64, :], av0[0:DK, :],
                                         rb_sb[:, 0, :])
                    avn1 = avn_pool.tile([64, 512], BF, name="avn1",
                                         tag="avn1")
                    nc.vector.tensor_mul(avn1, av1[0:DK, :], rb_sb[:, 1, :])
                    nc.sync.dma_start(out=AVT[p][64:128, :], in_=avn1)

                for p in range(npairs):
                    av0 = av_ps.tile([DK + 1, 512], FP, name="av0", tag="av")
                    av1 = av_ps.tile([DK + 1, 512], FP, name="av1", tag="av")
                    avs = (av0, av1)

                    def emit_av(kt, at, q0):
                        for hh in range(2):
                            nc.tensor.matmul(
                                avs[hh][:, q0:512],
                                V_sb[kt][:, 2 * p + hh, :],
                                at[:, hh, q0:512],
                                start=(kt == 0), stop=(kt == nkt - 1),
                                skip_group_check=True)

                    pend = []
                    for kt in range(nkt):
                        ksl = bass.ts(kt, 128)
                        # columns < j*128 of a diagonal tile are fully
                        # masked: skip in scores matmul / exp / mask / AV
                        j = kt - 4 * n if (causal and kt >= 4 * n) else 0
                        q0 = j * 128
                        ps_s = sc_ps.tile([128, 2, 512], FP, name="ps_s",
                                          tag="sc")
                        at = at_pool.tile([128, 2, 512], BF, name="at",
                                          tag="at")
                        for hh in range(2):
                            if kpad:
                                nc.tensor.matmul(
                                    ps_s[:, hh, q0:512],
                                    KTp[p][hh][:, ksl],
                                    QT[p][:, q0:512],
                                    start=True, stop=True)
                            else:
                                nc.tensor.matmul(
                                    ps_s[:, hh, q0:512],
                                    KT[p][hh * 64:(hh + 1) * 64, ksl],
                                    QT[p][hh * 64:(hh + 1) * 64, q0:512],
                                    start=True, stop=True,
                                    tile_position=(hh * 64, 0))
                        if half_exp:
                            # timing probe only: exp half the tile (WRONG
                            # results) to test whether S3 is ACT-bound
                            nc.scalar.activation(
                                at[:, 0:1, q0:512], ps_s[:, 0:1, q0:512],
                                mybir.ActivationFunctionType.Exp)
                            nc.vector.tensor_copy(at[:, 1, q0:512],
                                                  at[:, 0, q0:512])
                        else:
                            nc.scalar.activation(
                                at[:, :, q0:512], ps_s[:, :, q0:512],
                                mybir.ActivationFunctionType.Exp)
                        if causal and kt >= 4 * n and do_mask:
                            nc.vector.tensor_mul(
                                at[:, :, q0:q0 + 128], at[:, :, q0:q0 + 128],
                                bcast_mid(masks[0][:, 0:128], 2))
                        if kt == 0 and pending_norm[0] is not None:
                            pending_norm[0]()
                            pending_norm[0] = None
                        pend.append((kt, at, q0))
                        if len(pend) > PIPE:
                            emit_av(*pend.pop(0))
                        pace()
                    for item in pend:
                        emit_av(*item)
                    pending_norm[0] = (lambda p=p, a0=av0, a1=av1:
                                       emit_norm(p, a0, a1))
                if pending_norm[0] is not None:
                    pending_norm[0]()
                    pending_norm[0] = None
                while feed:
                    feed.pop(0)[1]()

            if micro is not None:
                emit_micro()
                return

            QT_cur, s2p = make_s2_parts(0, xT0)
            for c, g in s2p:
                g()
            f_prev = []
            for n in range(NCH):
                feed = []
                if n + 1 < NCH:
                    xT_next = emit_s1(n + 1)
                    QT_next, s2p_next = make_s2_parts(n + 1, xT_next)
                    feed += s2p_next
                else:
                    QT_next = None
                feed += f_prev
                if '3' in stages:
                    emit_s3(n, QT_cur, feed)
                else:
                    for c, g in feed:
                        g()
                f_prev = make_f_parts(n)
                QT_cur = QT_next
            for c, g in f_prev:
                g()

        if repeat > 1:
            with tc.For_i(0, repeat, 1):
                emit_kernel()
        else:
            emit_kernel()

    nc.compile()
    return nc


_NC_CACHE = {}


def _get_nc(seq_len, causal):
    key = (seq_len, causal)
    if key not in _NC_CACHE:
        _NC_CACHE[key] = build_bass(seq_len, causal)
    return _NC_CACHE[key]


def make_in_maps(x, w_q, b_q, w_k, w_v, w_o):
    """Per-core input dicts for the 8 cores (weights/x pre-cast to bf16).

    x is shipped pre-transposed and pre-tiled: [128, ND, T] bf16 with
    xt[p, j, t] = x[t, j*128 + p] (one clean 3D-AP DMA per chunk on device).
    """
    import ml_dtypes
    bf = ml_dtypes.bfloat16
    ND = D_MODEL // 128
    x_bf = np.ascontiguousarray(x).astype(bf)
    # [B, T, D] -> per-batch [D, T] -> [ND, 128, T] -> [128, ND, T]
    xt_bf = [np.ascontiguousarray(
        x_bf[b].T.reshape(ND, 128, -1).transpose(1, 0, 2))
        for b in range(x.shape[0])]
    wq_bf = w_q.astype(bf)
    wk_bf = w_k.astype(bf)
    wv_bf = w_v.astype(bf)
    wo_bf = w_o.astype(bf)
    in_maps = []
    for c in range(N_CORES):
        b, g = divmod(c, 2)
        sl = slice(g * DH, (g + 1) * DH)
        in_maps.append({
            "xbf": xt_bf[b],
            "wq": np.ascontiguousarray(wq_bf[:, sl]),
            "wk": np.ascontiguousarray(wk_bf[:, sl]),
            "wv": np.ascontiguousarray(wv_bf[:, sl]),
            "wo": np.ascontiguousarray(wo_bf[sl, :]),
            "bq": np.ascontiguousarray(
                b_q[sl].reshape(DH // 128, 128).T.astype(np.float32)),
        })
    return in_maps


def kernel(x, mask, w_q, b_q, w_k, b_k, w_v, b_v, w_o, b_o, _trace=False):
    x = np.asarray(x, dtype=np.float32)
    mask_np = np.asarray(mask).reshape(mask.shape[-2], mask.shape[-1])
    w_q, b_q = np.asarray(w_q, np.float32), np.asarray(b_q, np.float32)
    w_k = np.asarray(w_k, np.float32)
    w_v, b_v = np.asarray(w_v, np.float32), np.asarray(b_v, np.float32)
    w_o, b_o = np.asarray(w_o, np.float32), np.asarray(b_o, np.float32)

    T = x.shape[1]
    tril = np.tril(np.ones((T, T), dtype=mask_np.dtype))
    if np.array_equal(mask_np, tril):
        causal = True
    elif np.all(mask_np != 0):
        causal = False
    else:
        raise NotImplementedError("only causal or all-ones masks supported")

    nc = _get_nc(T, causal)
    in_maps = make_in_maps(x, w_q, b_q, w_k, w_v, w_o)
    res = run_bass_kernel_spmd(nc, in_maps, core_ids=list(range(N_CORES)),
                               trace=_trace)

    host_bias = (b_v @ w_o + b_o).astype(np.float32)
    out = np.empty((x.shape[0], T, D_MODEL), dtype=np.float32)
    for b in range(x.shape[0]):
        out[b] = res.results[2 * b]["out"] + res.results[2 * b + 1]["out"] \
            + host_bias
    kernel._last_result = res
    return out



# revision 3
# speedup vs baseline: 1.0809x; 1.0614x over previous
"""Multi-head causal attention (B=4, T=2048, D=1024, H=16) on 8 trn2 NeuronCores.

Sharding: core c handles batch b = c//2 and head-group g = c%2 (8 heads each).
Each core computes Q/K/V projections for its 8 heads, causal attention, and a
row-shard of the output projection; the host sums the two partial outputs per
batch (the "all-reduce") and adds the (b_v @ w_o + b_o) bias term.

Device algebra notes:
  - b_k drops out of softmax entirely (adds a per-query constant to scores).
  - b_v contributes exactly (b_v @ w_o) to every output row -> folded into the
    host-side bias along with b_o.
  - Scores are computed transposed ([k, q] layout) so that softmax sums ride
    free on the AV matmul (ones-column appended to V) and the AV output comes
    out as AV^T, which feeds the w_o matmul with no extra transposes.
  - x is shipped pre-transposed/pre-tiled from the host ([128, 8, T] bf16) so
    every device DMA is a clean 3D-AP contiguous transfer (no DMA transpose,
    which also removes the transpose||SBUF-SBUF DMA serialization hazard).
  - K^T is stored zero-padded per head-half (kpad) so the scores matmuls run
    with a full K=128 contraction (FWL-eligible weight loads) instead of two
    K=64 loads.
  - Softmax normalization: reciprocal on the 1-partition denominator row
    first, then a PE ones-outer-product broadcasts the reciprocal; the
    PSUM->SBUF bounce rides on the scalar engine, which has slack.
  - Independent projection/output-proj matmul work is paced into the
    attention inner loop in ~1us chunks (pace_tiles) so no engine sees a
    multi-us burst of foreign work.

  - All DMA layouts are packed host-side so every partition reads 4-8KB
    contiguous runs (sub-4KB HBM lines pay read-modify-write; bf16 output
    stores measured 40us slower for exactly that reason).

DMA rule: hardware DMA descriptors encode at most ONE semaphore wait, so every
DMA destination here is written exactly once per iteration; partition
broadcasts are done with PE outer products, not DMA.
"""

import math
from contextlib import ExitStack

import numpy as np

import concourse.bass as bass
import concourse.mybir as mybir
import concourse.tile as tile
from concourse import bacc
from concourse.bass_utils import run_bass_kernel_spmd


FP = mybir.dt.float32
BF = mybir.dt.bfloat16

D_MODEL = 1024
N_HEADS = 16
B_FULL, T_FULL = 4, 2048
DK = 64                    # head dim
HPC = 8                    # heads per core
DH = HPC * DK              # 512 head-dims per core
N_CORES = 8


def build_bass(seq_len=T_FULL, causal=True, repeat=1, stages='123F',
               pipe=3, do_mask=True, at_bufs=6, micro=None, micro_n=512,
               rb_on_act=True, half_exp=False, pace_tiles=4, kpad=True,
               out_bf=False, packed_dma=True):
    """Build the per-core Bass kernel (SPMD; same NEFF on all 8 cores).

    repeat > 1 wraps the whole kernel in a hardware loop — used only for
    benchmarking (amortizes host dispatch to time the kernel itself).
    """
    T = seq_len
    NT = T // 128             # t-tiles
    NCH = T // 512            # 512-wide t/q chunks
    ND = D_MODEL // 128       # d_model tiles (8)
    NM = DH // 128            # head-pair tiles (4)

    nc = bacc.Bacc("TRN2", target_bir_lowering=False, debug=False)
    # x arrives pre-transposed + pre-tiled on host: [128, ND, T] bf16 where
    # xt[p, j, t] = x[t, j*128 + p]
    if packed_dma:
        # per-partition-contiguous layouts: every DMA reads 4-8KB runs
        xbf_d = nc.dram_tensor("xbf", [128, T // 512, ND, 512], BF,
                               kind="ExternalInput")
        wq_d = nc.dram_tensor("wq", [128, ND, DH], BF, kind="ExternalInput")
        wk_d = nc.dram_tensor("wk", [128, ND, DH], BF, kind="ExternalInput")
        wv_d = nc.dram_tensor("wv", [128, ND, DH], BF, kind="ExternalInput")
        wo_d = nc.dram_tensor("wo", [128, NM, D_MODEL], BF,
                              kind="ExternalInput")
    else:
        xbf_d = nc.dram_tensor("xbf", [128, ND, T], BF,
                               kind="ExternalInput")
        wq_d = nc.dram_tensor("wq", [D_MODEL, DH], BF, kind="ExternalInput")
        wk_d = nc.dram_tensor("wk", [D_MODEL, DH], BF, kind="ExternalInput")
        wv_d = nc.dram_tensor("wv", [D_MODEL, DH], BF, kind="ExternalInput")
        wo_d = nc.dram_tensor("wo", [DH, D_MODEL], BF, kind="ExternalInput")
    bq_d = nc.dram_tensor("bq", [128, DH // 128], FP, kind="ExternalInput")
    out_d = nc.dram_tensor("out", [T, D_MODEL], BF if out_bf else FP,
                           kind="ExternalOutput")

    with ExitStack() as ctx:
        tc = ctx.enter_context(tile.TileContext(nc))
        persist = ctx.enter_context(tc.tile_pool(name="persist", bufs=1))
        qt_pool = ctx.enter_context(tc.tile_pool(name="qt", bufs=3))
        xt_pool = ctx.enter_context(tc.tile_pool(name="xt", bufs=3))
        at_pool = ctx.enter_context(tc.tile_pool(name="atp", bufs=at_bufs))
        rec_pool = ctx.enter_context(tc.tile_pool(name="rec", bufs=2))
        avn_pool = ctx.enter_context(tc.tile_pool(name="avn", bufs=2))
        out_pool = ctx.enter_context(tc.tile_pool(name="outp", bufs=3))
        mm_ps = ctx.enter_context(tc.tile_pool(name="mmps", bufs=2, space="PSUM"))
        sc_ps = ctx.enter_context(tc.tile_pool(name="scps", bufs=2, space="PSUM"))
        av_ps = ctx.enter_context(tc.tile_pool(name="avps", bufs=2, space="PSUM"))

        def emit_kernel():
            # ---- constants -------------------------------------------------
            ones_bf = persist.tile([1, DK], BF, name="ones_bf", tag="ones_bf")
            nc.gpsimd.memset(ones_bf, 1.0)

            masks = []
            if causal:
                # one [128,128] lower-triangle tile: keep where col >= row.
                # Every diagonal k-tile needs masking only on its first 128
                # columns (cols beyond are fully unmasked, earlier cols are
                # fully masked and already skipped via q0).
                m = persist.tile([128, 128], BF, name="mask01", tag="mask01")
                nc.gpsimd.memset(m, 1.0)
                nc.gpsimd.affine_select(
                    out=m, in_=m, compare_op=mybir.AluOpType.is_ge,
                    fill=0.0, base=0, pattern=[[1, 128]],
                    channel_multiplier=-1)
                masks = [m] * 4

            def bcast_mid(ap, nmid):
                return bass.AP(tensor=ap.tensor, offset=ap.offset,
                               ap=[list(ap.ap[0]), [0, nmid], list(ap.ap[1])])

            # x chunk 0 DMA first so S2(0) can start as early as possible
            xt0 = xt_pool.tile([128, ND, 512], BF, name="xTt", tag="xTt")
            if packed_dma:
                nc.sync.dma_start(out=xt0, in_=xbf_d[:, 0, :, :])
            else:
                nc.sync.dma_start(out=xt0, in_=xbf_d[:, :, 0:512])
            xT0 = [xt0[:, j, :] for j in range(ND)]

            # ---- weights: one 3D-AP HWDGE DMA per matrix (host pre-cast) ---
            def load_weight_bf(dram, n_tiles, cols, label):
                wsb = persist.tile([128, n_tiles, cols], BF, name=f"w_{label}",
                                   tag=f"w_{label}")
                if packed_dma:
                    nc.sync.dma_start(out=wsb, in_=dram[:, :, :])
                else:
                    nc.sync.dma_start(
                        out=wsb,
                        in_=dram.ap().rearrange("(j p) c -> p j c", p=128))
                return [wsb[:, j, :] for j in range(n_tiles)]

            wk_bf = load_weight_bf(wk_d, ND, DH, "k")
            wq_bf = load_weight_bf(wq_d, ND, DH, "q")
            wv_bf = load_weight_bf(wv_d, ND, DH, "v")

            bq_sb = persist.tile([128, NM], FP, name="bq_sb", tag="bq_sb")
            nc.sync.dma_start(out=bq_sb, in_=bq_d[:, :])

            wo_bf = load_weight_bf(wo_d, NM, D_MODEL, "o")

            # ---- persistent per-core tensors ------------------------------
            # KT[mt]: [128, T] bf16, rows = head-dim (pair mt: heads 2mt,2mt+1)
            # kpad: per head-half tiles with the other 64 rows zeroed, so the
            # scores matmuls are full K=128 (FWL-eligible; no K=64 LDW
            # penalty). rhs is then the full 128-row QT (the other head's
            # rows meet zero weights).
            if kpad:
                KTp = [[persist.tile([128, T], BF, name=f"KT{mt}h{hh}",
                                     tag=f"KT{mt}h{hh}") for hh in range(2)]
                       for mt in range(NM)]
                for mt in range(NM):
                    nc.gpsimd.memset(KTp[mt][0][64:128, :], 0.0)
                    nc.gpsimd.memset(KTp[mt][1][0:64, :], 0.0)
                KT = None
            else:
                KT = [persist.tile([128, T], BF, name=f"KT{mt}",
                                   tag=f"KT{mt}") for mt in range(NM)]
            # V natural layout with ones column: [128 t, 8 heads, 64+1]
            V_sb = [persist.tile([128, HPC, DK + 1], BF, name=f"V{i}", tag=f"V{i}")
                    for i in range(NT)]
            for i in range(NT):
                nc.gpsimd.memset(V_sb[i][:, :, DK], 1.0)
            # AV^T, write-once (DMA target for the h1 partition shift)
            AVT_all = [[persist.tile([128, 512], BF, name=f"AVT{n}_{p}",
                                     tag=f"AVT{n}_{p}") for p in range(NM)]
                       for n in range(NCH)]

            def emit_micro():
                """Microbench streams using the real kernel's tiles/pools."""
                if micro in ("mm_same_w", "mm_alt_w"):
                    pss = [mm_ps.tile([128, 512], FP, name=f"psu{i}",
                                      tag="mm") for i in range(2)]
                    for i in range(micro_n):
                        j = 0 if micro == "mm_same_w" else i % ND
                        nc.tensor.matmul(pss[i % 2], wq_bf[j][:, 0:128],
                                         wq_bf[1], start=True, stop=True,
                                         skip_group_check=True)
                    consume = pss[0][:, 0:512]
                elif micro in ("mm_k64_pair", "mm_k64_serial"):
                    sss = [sc_ps.tile([128, 2, 512], FP, name=f"ps_su{i}",
                                      tag="sc") for i in range(2)]
                    for i in range(micro_n // 2):
                        for hh in range(2):
                            if micro == "mm_k64_pair":
                                nc.tensor.matmul(
                                    sss[i % 2][:, hh, :],
                                    wk_bf[0][hh * 64:(hh + 1) * 64, 0:128],
                                    wk_bf[1][hh * 64:(hh + 1) * 64, 0:512],
                                    start=True, stop=True,
                                    skip_group_check=True,
                                    tile_position=(hh * 64, 0))
                            else:
                                nc.tensor.matmul(
                                    sss[i % 2][:, hh, :],
                                    wk_bf[0][0:64, 0:128],
                                    wk_bf[1][0:64, 0:512],
                                    start=True, stop=True,
                                    skip_group_check=True,
                                    tile_position=(0, 0))
                    consume = sss[0][:, 0, :]
                elif micro == "mm_n1024":
                    ps2b = [sc_ps.tile([128, 2, 512], FP, name=f"psw{i}",
                                       tag="sc") for i in range(2)]
                    for i in range(micro_n // 2):
                        nc.tensor.matmul(
                            ps2b[i % 2].rearrange("p a b -> p (a b)"),
                            wq_bf[i % ND][:, 0:128], wo_bf[0],
                            start=True, stop=True, skip_group_check=True)
                    consume = ps2b[0][:, 0, :]
                elif micro == "mm_m65":
                    avs = [av_ps.tile([DK + 1, 512], FP, name=f"avu{i}",
                                      tag="av") for i in range(2)]
                    for i in range(micro_n):
                        nc.tensor.matmul(avs[i % 2], wv_bf[0][:, 0:65],
                                         wk_bf[1][:, 0:512],
                                         start=True, stop=True,
                                         skip_group_check=True)
                    consume = avs[0][0:64, :]
                elif micro in ("act_exp", "act_exp1", "dve_copy_ps"):
                    ps_s = sc_ps.tile([128, 2, 512], FP, name="ps_su",
                                      tag="sc")
                    at = at_pool.tile([128, 2, 512], BF, name="atu",
                                      tag="at")
                    for hh in range(2):
                        nc.tensor.matmul(ps_s[:, hh, :], wq_bf[0][:, 0:128],
                                         wq_bf[1], start=True, stop=True,
                                         skip_group_check=True)
                    for i in range(micro_n):
                        if micro == "act_exp":
                            nc.scalar.activation(
                                at, ps_s, mybir.ActivationFunctionType.Exp)
                        elif micro == "act_exp1":
                            nc.scalar.activation(
                                at[:, 0, :], ps_s[:, 0, :],
                                mybir.ActivationFunctionType.Exp)
                        else:
                            nc.vector.tensor_copy(at[:, 0, :], ps_s[:, 0, :])
                    consume = at[:, 0, 0:512]
                elif micro == "dve_mul":
                    at = at_pool.tile([128, 2, 512], BF, name="atu",
                                      tag="at")
                    at2 = at_pool.tile([128, 2, 512], BF, name="atu2",
                                      tag="at")
                    nc.gpsimd.memset(at2, 1.0)
                    for i in range(micro_n):
                        nc.vector.tensor_mul(
                            at, at2, bcast_mid(masks[0], 2))
                    consume = at[:, 0, 0:512]
                else:
                    raise ValueError(micro)
                osb = out_pool.tile([128, D_MODEL], FP, name="osb",
                                    tag="osb")
                npart = consume.ap[0][1] if hasattr(consume, "ap") else 128
                nc.vector.tensor_copy(osb[0:npart, 0:512], consume)
                nc.sync.dma_start(out=out_d[0:128, :], in_=osb)

            # ---- main streamed loop over 512-wide chunks -------------------
            # S2(n+1) and F(n-1) matmul groups are fed into S3(n)'s pair
            # boundaries: the per-engine queues are in-order, so S3's
            # exp-latency bubbles can only be filled by positionally
            # interleaving independent work into its instruction stream.

            def emit_s1(n):
                xt = xt_pool.tile([128, ND, 512], BF, name="xTt", tag="xTt")
                if packed_dma:
                    nc.sync.dma_start(out=xt, in_=xbf_d[:, n, :, :])
                else:
                    nc.sync.dma_start(out=xt,
                                      in_=xbf_d[:, :, n * 512:(n + 1) * 512])
                return [xt[:, j, :] for j in range(ND)]

            def make_s2_parts(n, xT):
                """Projection work for chunk n as (est_ns, emit_fn) parts.

                Each q/k/v group (8 accumulating MMs + a vector op) is split
                into two 4-MM halves so the S3 pacer can interleave at ~1us
                granularity. The halves share one PSUM tile via `state`.
                """
                csl = bass.ts(n, 512)
                QT = [qt_pool.tile([128, 512], BF, name=f"QT{mt}",
                                   tag=f"QT{mt}") for mt in range(NM)]
                parts = []
                state = {}

                def half(kind, mt, j0):
                    msl = bass.ts(mt, 128)
                    if j0 == 0:
                        ps = mm_ps.tile([128, 512], FP, name=f"ps{kind}",
                                        tag="mm")
                        state[(kind, mt)] = ps
                    else:
                        ps = state.pop((kind, mt))
                    for j in range(j0, j0 + 4):
                        if kind == 'v':
                            nc.tensor.matmul(
                                ps, xT[j][:, mt * 128:(mt + 1) * 128],
                                wv_bf[j], start=(j == 0),
                                stop=(j == ND - 1))
                        else:
                            w = wq_bf if kind == 'q' else wk_bf
                            nc.tensor.matmul(ps, w[j][:, msl], xT[j],
                                             start=(j == 0),
                                             stop=(j == ND - 1))
                    if j0 + 4 < ND:
                        return
                    if kind == 'q':
                        nc.vector.tensor_scalar_add(QT[mt], ps,
                                                    bq_sb[:, mt:mt + 1])
                    elif kind == 'k':
                        # fold the 1/sqrt(dk) score scale into K^T
                        sc = 1.0 / math.sqrt(DK)
                        if kpad:
                            nc.vector.tensor_scalar_mul(
                                KTp[mt][0][0:64, csl], ps[0:64, :], sc)
                            nc.vector.tensor_scalar_mul(
                                KTp[mt][1][64:128, csl], ps[64:128, :], sc)
                        else:
                            nc.vector.tensor_scalar_mul(KT[mt][:, csl], ps,
                                                        sc)
                    else:
                        i = 4 * n + mt
                        nc.vector.tensor_copy(
                            V_sb[i][:, :, 0:DK],
                            ps.rearrange("p (h d) -> p h d", h=HPC))

                if '2' in stages:
                    for mt in range(NM):
                        for kind in ('k', 'q'):
                            for j0 in (0, 4):
                                parts.append((990, lambda kind=kind, mt=mt,
                                              j0=j0: half(kind, mt, j0)))
                    for il in range(4):
                        for j0 in (0, 4):
                            parts.append((990, lambda il=il, j0=j0:
                                          half('v', il, j0)))
                return QT, parts

            def make_f_parts(n):
                AVT = AVT_all[n]
                parts = []
                osbs = {}

                def fgroup(il, cc):
                    i = 4 * n + il
                    isl = bass.ts(il, 128)
                    if il not in osbs:
                        osbs[il] = out_pool.tile([128, D_MODEL],
                                                 BF if out_bf else FP,
                                                 name="osb", tag="osb")
                    osb = osbs[il]
                    ps = mm_ps.tile([128, 512], FP, name="pso", tag="mm")
                    for dk in range(NM):
                        nc.tensor.matmul(
                            ps, AVT[dk][:, isl],
                            wo_bf[dk][:, cc * 512:(cc + 1) * 512],
                            start=(dk == 0), stop=(dk == NM - 1))
                    if out_bf:
                        with nc.allow_low_precision(
                                reason="bf16 partial-output store adds "
                                       "~0.4% quantization, inside budget"):
                            nc.vector.tensor_copy(
                                osb[:, cc * 512:(cc + 1) * 512], ps)
                    else:
                        nc.vector.tensor_copy(
                            osb[:, cc * 512:(cc + 1) * 512], ps)
                    if cc == 1:
                        nc.sync.dma_start(
                            out=out_d[i * 128:(i + 1) * 128, :], in_=osb)

                if 'F' in stages:
                    for il in range(4):
                        for cc in range(2):
                            parts.append((1030, lambda il=il, cc=cc:
                                          fgroup(il, cc)))
                return parts

            def emit_s3(n, QT, feed):
                AVT = AVT_all[n]
                nkt = 4 * n + 4 if causal else NT
                PIPE = pipe
                pending_norm = [None]
                npairs = NM if '3' in stages else 0

                # proportional pacer: spread the independent S2/F matmul work
                # evenly across this chunk's kt tiles so PE never idles while
                # ACT streams exps, and ACT never idles through a feed burst.
                feed_total = sum(c for c, _ in feed)
                total_tiles = max(1, nkt * max(npairs, 1))
                done = [0.0]
                tcount = [0]

                def pace():
                    tcount[0] += 1
                    if pace_tiles > 0 and tcount[0] % pace_tiles != 0:
                        return
                    target = feed_total * tcount[0] / total_tiles
                    while feed and done[0] < target:
                        c, g = feed.pop(0)
                        g()
                        done[0] += c

                def emit_norm(p, av0, av1):
                    # reciprocal on the 1-partition denominator rows, then a
                    # PE ones-outer-product broadcast of the RECIPROCAL, and
                    # an ACT copy bounce PSUM->SBUF (DVE is the scarcer
                    # engine; ACT has slack).
                    rec_sb = rec_pool.tile([1, 2, 512], BF, name="rec_sb",
                                           tag="den_bf")
                    with nc.allow_low_precision(
                            reason="bf16 1/den adds ~0.4% to the softmax "
                                   "denominator, well inside the error "
                                   "budget"):
                        nc.vector.reciprocal(rec_sb[:, 0, :],
                                             av0[DK:DK + 1, :])
                        nc.vector.reciprocal(rec_sb[:, 1, :],
                                             av1[DK:DK + 1, :])
                    rb_sb = rec_pool.tile([DK, 2, 512], BF, name="rb_sb",
                                          tag="rb_sb")
                    for hh in range(2):
                        rb = mm_ps.tile([DK, 512], FP, name=f"rb{hh}",
                                        tag="mm")
                        nc.tensor.matmul(rb, ones_bf, rec_sb[:, hh, :],
                                         start=True, stop=True)
                        if rb_on_act:
                            nc.scalar.copy(rb_sb[:, hh, :], rb)
                        else:
                            nc.vector.tensor_copy(rb_sb[:, hh, :], rb)
                    nc.vector.tensor_mul(AVT[p][0:64, :], av0[0:DK, :],
                                         rb_sb[:, 0, :])
                    avn1 = avn_pool.tile([64, 512], BF, name="avn1",
                                         tag="avn1")
                    nc.vector.tensor_mul(avn1, av1[0:DK, :], rb_sb[:, 1, :])
                    nc.sync.dma_start(out=AVT[p][64:128, :], in_=avn1)

                for p in range(npairs):
                    av0 = av_ps.tile([DK + 1, 512], FP, name="av0", tag="av")
                    av1 = av_ps.tile([DK + 1, 512], FP, name="av1", tag="av")
                    avs = (av0, av1)

                    def emit_av(kt, at, q0):
                        for hh in range(2):
                            nc.tensor.matmul(
                                avs[hh][:, q0:512],
                                V_sb[kt][:, 2 * p + hh, :],
                                at[:, hh, q0:512],
                                start=(kt == 0), stop=(kt == nkt - 1),
                                skip_group_check=True)

                    pend = []
                    for kt in range(nkt):
                        ksl = bass.ts(kt, 128)
                        # columns < j*128 of a diagonal tile are fully
                        # masked: skip in scores matmul / exp / mask / AV
                        j = kt - 4 * n if (causal and kt >= 4 * n) else 0
                        q0 = j * 128
                        ps_s = sc_ps.tile([128, 2, 512], FP, name="ps_s",
                                          tag="sc")
                        at = at_pool.tile([128, 2, 512], BF, name="at",
                                          tag="at")
                        for hh in range(2):
                            if kpad:
                                nc.tensor.matmul(
                                    ps_s[:, hh, q0:512],
                                    KTp[p][hh][:, ksl],
                                    QT[p][:, q0:512],
                                    start=True, stop=True)
                            else:
                                nc.tensor.matmul(
                                    ps_s[:, hh, q0:512],
                                    KT[p][hh * 64:(hh + 1) * 64, ksl],
                                    QT[p][hh * 64:(hh + 1) * 64, q0:512],
                                    start=True, stop=True,
                                    tile_position=(hh * 64, 0))
                        if half_exp:
                            # timing probe only: exp half the tile (WRONG
                            # results) to test whether S3 is ACT-bound
                            nc.scalar.activation(
                                at[:, 0:1, q0:512], ps_s[:, 0:1, q0:512],
                                mybir.ActivationFunctionType.Exp)
                            nc.vector.tensor_copy(at[:, 1, q0:512],
                                                  at[:, 0, q0:512])
                        else:
                            nc.scalar.activation(
                                at[:, :, q0:512], ps_s[:, :, q0:512],
                                mybir.ActivationFunctionType.Exp)
                        if causal and kt >= 4 * n and do_mask:
                            nc.vector.tensor_mul(
                                at[:, :, q0:q0 + 128], at[:, :, q0:q0 + 128],
                                bcast_mid(masks[0][:, 0:128], 2))
                        if kt == 0 and pending_norm[0] is not None:
                            pending_norm[0]()
                            pending_norm[0] = None
                        pend.append((kt, at, q0))
                        if len(pend) > PIPE:
                            emit_av(*pend.pop(0))
                        pace()
                    for item in pend:
                        emit_av(*item)
                    pending_norm[0] = (lambda p=p, a0=av0, a1=av1:
                                       emit_norm(p, a0, a1))
                if pending_norm[0] is not None:
                    pending_norm[0]()
                    pending_norm[0] = None
                while feed:
                    feed.pop(0)[1]()

            if micro is not None:
                emit_micro()
                return

            QT_cur, s2p = make_s2_parts(0, xT0)
            for c, g in s2p:
                g()
            f_prev = []
            for n in range(NCH):
                feed = []
                if n + 1 < NCH:
                    xT_next = emit_s1(n + 1)
                    QT_next, s2p_next = make_s2_parts(n + 1, xT_next)
                    feed += s2p_next
                else:
                    QT_next = None
                feed += f_prev
                if '3' in stages:
                    emit_s3(n, QT_cur, feed)
                else:
                    for c, g in feed:
                        g()
                f_prev = make_f_parts(n)
                QT_cur = QT_next
            for c, g in f_prev:
                g()

        if repeat > 1:
            with tc.For_i(0, repeat, 1):
                emit_kernel()
        else:
            emit_kernel()

    nc.compile()
    return nc


_NC_CACHE = {}


def _get_nc(seq_len, causal):
    key = (seq_len, causal)
    if key not in _NC_CACHE:
        _NC_CACHE[key] = build_bass(seq_len, causal)
    return _NC_CACHE[key]


def make_in_maps(x, w_q, b_q, w_k, w_v, w_o, packed_dma=True):
    """Per-core input dicts for the 8 cores (weights/x pre-cast to bf16).

    x is shipped pre-transposed and pre-tiled: [128, ND, T] bf16 with
    xt[p, j, t] = x[t, j*128 + p] (one clean 3D-AP DMA per chunk on device).
    """
    import ml_dtypes
    bf = ml_dtypes.bfloat16
    ND = D_MODEL // 128
    x_bf = np.ascontiguousarray(x).astype(bf)
    # [B, T, D] -> per-batch [D, T] -> [ND, 128, T] -> [128, ND, T]
    xt_bf = [np.ascontiguousarray(
        x_bf[b].T.reshape(ND, 128, -1).transpose(1, 0, 2))
        for b in range(x.shape[0])]
    if packed_dma:
        # [128, ND, T] -> [128, NCH, ND, 512]: 8KB contiguous per
        # partition per chunk
        T = x.shape[1]
        xt_bf = [np.ascontiguousarray(
            xb.reshape(128, ND, T // 512, 512).transpose(0, 2, 1, 3))
            for xb in xt_bf]

    def pack_w(w):  # [(j p), c] -> [128 p, j, c], per-partition contiguous
        jc = w.shape[0] // 128
        return np.ascontiguousarray(
            w.reshape(jc, 128, w.shape[1]).transpose(1, 0, 2))
    wq_bf = w_q.astype(bf)
    wk_bf = w_k.astype(bf)
    wv_bf = w_v.astype(bf)
    wo_bf = w_o.astype(bf)
    in_maps = []
    for c in range(N_CORES):
        b, g = divmod(c, 2)
        sl = slice(g * DH, (g + 1) * DH)
        wq_c = np.ascontiguousarray(wq_bf[:, sl])
        wk_c = np.ascontiguousarray(wk_bf[:, sl])
        wv_c = np.ascontiguousarray(wv_bf[:, sl])
        wo_c = np.ascontiguousarray(wo_bf[sl, :])
        if packed_dma:
            wq_c, wk_c, wv_c, wo_c = map(pack_w, (wq_c, wk_c, wv_c, wo_c))
        in_maps.append({
            "xbf": xt_bf[b],
            "wq": wq_c,
            "wk": wk_c,
            "wv": wv_c,
            "wo": wo_c,
            "bq": np.ascontiguousarray(
                b_q[sl].reshape(DH // 128, 128).T.astype(np.float32)),
        })
    return in_maps


def kernel(x, mask, w_q, b_q, w_k, b_k, w_v, b_v, w_o, b_o, _trace=False):
    x = np.asarray(x, dtype=np.float32)
    mask_np = np.asarray(mask).reshape(mask.shape[-2], mask.shape[-1])
    w_q, b_q = np.asarray(w_q, np.float32), np.asarray(b_q, np.float32)
    w_k = np.asarray(w_k, np.float32)
    w_v, b_v = np.asarray(w_v, np.float32), np.asarray(b_v, np.float32)
    w_o, b_o = np.asarray(w_o, np.float32), np.asarray(b_o, np.float32)

    T = x.shape[1]
    tril = np.tril(np.ones((T, T), dtype=mask_np.dtype))
    if np.array_equal(mask_np, tril):
        causal = True
    elif np.all(mask_np != 0):
        causal = False
    else:
        raise NotImplementedError("only causal or all-ones masks supported")

    nc = _get_nc(T, causal)
    in_maps = make_in_maps(x, w_q, b_q, w_k, w_v, w_o)
    res = run_bass_kernel_spmd(nc, in_maps, core_ids=list(range(N_CORES)),
                               trace=_trace)

    host_bias = (b_v @ w_o + b_o).astype(np.float32)
    out = np.empty((x.shape[0], T, D_MODEL), dtype=np.float32)
    for b in range(x.shape[0]):
        out[b] = res.results[2 * b]["out"] + res.results[2 * b + 1]["out"] \
            + host_bias
    kernel._last_result = res
    return out

